# revision 24
# baseline (speedup 1.0000x reference)
"""Expected Calibration Error (ECE) kernel for Trainium2, 8 NeuronCores.

Problem: inputs [2e6, 128] f32 row-probabilities, targets [2e6] int64.
  conf_i = max_c inputs[i, c];  pred_i = argmax_c inputs[i, c]
  bin_i  = bucketize(conf_i, linspace(0, 1, 11), right=True) - 1
  ECE    = sum_b |corr_sum[b] - conf_sum[b]| / N

Strategy (data-parallel over rows, 250k rows per core):
  The confidence per row is a pure paged max: one custom DVE op per
  2048-row supertile streams in0 = [128, 16, 128] (16 rows per
  partition) and computes an inclusive MAX-scan that RESETS at each
  row (page) boundary via a hand-crafted SUB_DIM_DONE step uop.  The
  page-end element of the scan is that row's conf; the (otherwise
  idle) Scalar engine extracts column 127 of each page into a packed
  conf tile.  This amortizes the ~115ns fixed DVE instruction cost
  over 2048 elements instead of 128, cutting Vector busy ~2x below
  the HBM roofline so the kernel is DMA-bound (the memory minimum).

  Correctness per row needs no argmax: the host precomputes
  tprob[i] = inputs[i, targets[i]] (gather) and ships it in the same
  [128, NTG] column layout as conf; on-chip correct = [tprob >= conf]
  (equality iff the target attains the row max; exact-f32-tie rows are
  ~1e-5 of data and ignorable).

  Binning: G_b = [conf >= e_b] for the 10 edges gives cumulative
  per-bin sums via tiny TensorE matmuls psum[2,10] += [conf,correct]^T
  x G per column; host differences adjacent bins and finishes
  sum |corr - conf| / N.

Sharding: rows split evenly, 250,000 per core = 122 supertiles x 16
tiles (p-major contiguous DMA) + 1 plain tile + 1 partial 16-row tile.
"""

import numpy as np

N = 2_000_000
C = 128
NCORES = 8
ROWS = N // NCORES            # 250_000
NST = 122                     # supertiles of 16 pages (2048 rows each)
ST_PAGES = 16
ST_ROWS = 128 * ST_PAGES      # 2048
NT_MAIN = NST * ST_PAGES      # 1952 conf columns via supertiles
# column 1952: full 128-row tile; column 1953: 16-row partial tile
NTG = NT_MAIN + 2             # 1954 conf columns
PARTIAL_ROWS = ROWS - NST * ST_ROWS - 128  # 16

# conf columns per epilogue/matmul chunk; chunk boundaries must be
# multiples of 16 (a supertile writes 16 columns in one op); smaller
# chunks at the end shrink the serial tail after the last supertile
CHUNK_SIZES = [256] * 7 + [64, 64, 34]
assert sum(CHUNK_SIZES) == NTG
CHUNK_STARTS = [sum(CHUNK_SIZES[:i]) for i in range(len(CHUNK_SIZES))]
NCHUNKS = len(CHUNK_SIZES)
for _s in CHUNK_STARTS:
    assert _s % 16 == 0

# columns per matmul: each matmul contracts GROUP columns at once into
# a [2*GROUP, 10*GROUP] PSUM accumulator (the host reads the diagonal
# sub-blocks); 16x fewer TensorE instructions than per-column matmuls,
# whose instruction-stream fetch DMAs congest queue 0
GROUP = 16
# last chunk padded to a GROUP multiple; pad cols are zeroed
CHUNK_PADS = [-(-sz // GROUP) * GROUP for sz in CHUNK_SIZES]
NGROUPS = sum(p // GROUP for p in CHUNK_PADS)

EDGES = [float(e) for e in np.linspace(0.0, 1.0, 11).astype(np.float32)[:10]]

_f32 = np.float32


def _paged_max_ref(in0, in1, c0, c1, c2):
    a = np.asarray(in0, np.float32)
    b = np.asarray(in1, np.float32).reshape(a.shape)
    m = np.maximum(a, b)
    if m.ndim == 2:
        m = m[:, None, :]
    return np.maximum.accumulate(m, axis=-1)


def _register_op():
    import copy

    from concourse.dve_ops import (
        DveOp,
        OPS,
        CUSTOM_DVE_SPECS,
        _SUB_OPCODE_FOR_NAME,
        _CUSTOM_DVE_ROW_BASE,
        _COMPILE_CACHE,
    )
    from concourse.dve_spec import Spec, Src0, Src1, MaxNeg, maxx, scan, lower, AluOp
    from concourse.dve_uop import DveOpSpec, Trigger, AluInp, ENABLE

    name = "ECE_PAGED_MAX_ANT"
    if name in _SUB_OPCODE_FOR_NAME:
        return next(op for op in OPS if op.name == name)

    # two streams (half a row each) so the DVE eats 2 classes/cycle
    spec = Spec(
        body=scan(AluOp.MAX, maxx(Src0, Src1), init=MaxNeg),
        reference=_paged_max_ref,
    )
    row = _CUSTOM_DVE_ROW_BASE + len(OPS)
    assert row < 0x20
    _SUB_OPCODE_FOR_NAME[name] = row

    shas = {}
    for ver in ("v3", "v4"):
        try:
            seed, steady = lower(spec, ver=ver)
        except Exception:
            continue
        # locate the scan stage (MAX with same-stage CURR feedback)
        ss = next(
            i
            for i, b in enumerate(steady.datapath_config)
            if b.op == AluOp.MAX and b.alu_src0 == AluInp.CURR_ALU_OUT
        )
        src0_inp = steady.datapath_config[ss].alu_src1  # the Src0 lane
        # steady: on each page boundary jump to the step uop (index 2)
        steady.trigger = (
            Trigger.SRC_TENSOR_DONE,
            Trigger.SUB_DIM_DONE,
            Trigger.NONE,
        )
        steady.next_uop = (0, 2, 0)
        # step: first element of a new page resets the running max to
        # that element (BYPASS of Src0 instead of MAX with the carry)
        step = copy.deepcopy(steady)
        dp = step.datapath_config[ss]
        dp.op = AluOp.BYPASS
        dp.alu_src0 = src0_inp
        dp.alu_src1 = src0_inp
        step.trigger = (
            Trigger.SRC_TENSOR_DONE,
            Trigger.SUB_DIM_DONE,
            Trigger.COUNT,
        )
        step.next_uop = (0, 2, 1)
        step.repeat_count = 1
        uops = [seed, steady, step]
        dspec = DveOpSpec(name=name, opcode=row, uops=uops, rd1_en=True)
        dspec.validate(ver)
        _COMPILE_CACHE[(name, ver)] = dspec
        shas[ver] = dspec.sha(ver)
    op = DveOp(name, spec, subdim=True, uops_sha=shas)
    OPS.append(op)
    CUSTOM_DVE_SPECS[name] = spec
    return op


_NC_CACHE = None


def _build_bass():
    global _NC_CACHE
    if _NC_CACHE is not None:
        return _NC_CACHE

    import concourse.bacc as bacc
    import concourse.tile as tile
    from concourse import mybir

    pmax_op = _register_op()

    nc = bacc.Bacc()
    f32 = mybir.dt.float32
    x = nc.dram_tensor("x", [ROWS, C], f32, kind="ExternalInput")
    tp = nc.dram_tensor("tp", [128, NTG], f32, kind="ExternalInput")
    out = nc.dram_tensor("out", [2 * GROUP, 10 * GROUP], f32, kind="ExternalOutput")

    with tile.TileContext(nc) as tc:
        with (
            tc.tile_pool(name="persist", bufs=1) as persist,
            tc.tile_pool(name="inbuf", bufs=6) as inbuf,
            tc.tile_pool(name="tailbuf", bufs=1) as tailbuf,
            tc.tile_pool(name="mxbuf", bufs=3) as mxbuf,
            tc.tile_pool(name="decbuf", bufs=3) as decbuf,
            tc.tile_pool(name="psum", bufs=1, space="PSUM") as psumpool,
        ):
            # [128, NTG] target-prob tile, loaded as one 1MB transfer so
            # the DMA splits evenly across all 16 queues (chunked 128KB
            # loads all land on one queue and make it a straggler)
            tp_tile = persist.tile([128, NTG], f32, name="tpt", tag="tpt")

            # per-chunk [conf, correct] tiles, group-major so each matmul
            # reads a contiguous [128, 2*GROUP] block: (p, group, a, j);
            # conf (a=0) is written one group at a time by the extracts
            cc_tiles = [
                persist.tile(
                    [128, CHUNK_PADS[c] // GROUP, 2, GROUP],
                    f32,
                    name=f"cc{c}",
                    tag=f"cc{c}",
                )
                for c in range(NCHUNKS)
            ]
            # partial-tile column: partitions 16.. of the last column are
            # never written, and pad columns are never written; conf=0
            # there (with tprob=-1) zeroes their contribution
            nc.vector.memset(cc_tiles[-1][:], 0.0)

            psum = psumpool.tile([2 * GROUP, 10 * GROUP], f32)

            x_ap = x[:]
            xr = x_ap[: NST * ST_ROWS, :].rearrange(
                "(s p k) c -> s p k c", s=NST, p=128, k=16
            )

            import bisect

            def conf_dst(j, npages, nparts=128):
                """conf destination AP for columns [j, j+npages)."""
                c = bisect.bisect_right(CHUNK_STARTS, j) - 1
                l = j - CHUNK_STARTS[c]
                gi, jo = l // GROUP, l % GROUP
                assert jo + npages <= GROUP
                return cc_tiles[c][:nparts, gi, 0, jo : jo + npages]

            H = C // 2

            def emit_pmax(xt_ap, j, npages, scratch, nparts=128):
                """paged-max over xt [nparts, npages, C]; conf -> cols j.."""
                nc.vector._custom_dve(
                    pmax_op,
                    out=scratch[:nparts, :npages, :],
                    in0=xt_ap[:, :, :H],
                    in1=xt_ap[:, :, H:],
                )
                # page-end element of the scan = the page max
                nc.scalar.activation(
                    out=conf_dst(j, npages, nparts),
                    in_=scratch[:nparts, :npages, H - 1],
                    func=mybir.ActivationFunctionType.Copy,
                    bias=0.0,
                    scale=1.0,
                )

            group_base = [sum(p // GROUP for p in CHUNK_PADS[:c]) for c in range(NCHUNKS)]

            def emit_chunk_epilogue(c):
                ncols = CHUNK_SIZES[c]
                npad = CHUNK_PADS[c]
                ngrp = npad // GROUP
                nfull = ncols // GROUP  # groups fully covered by real cols
                a = CHUNK_STARTS[c]
                cc = cc_tiles[c]
                if npad != ncols:
                    g = decbuf.tile(
                        [128, ngrp, 10, GROUP], f32, name=f"g2_{c}", tag=f"g2_{c}", bufs=1
                    )
                    nc.vector.memset(g[:], 0.0)
                else:
                    g = decbuf.tile([128, 16, 10, GROUP], f32, name="g", tag="g")
                tpr = tp_tile[:, a : a + nfull * GROUP].rearrange(
                    "p (g j) -> p g j", g=nfull
                )
                # correct flag: [tprob >= conf]
                nc.vector.tensor_tensor(
                    out=cc[:, :nfull, 1, :],
                    in0=tpr,
                    in1=cc[:, :nfull, 0, :],
                    op=mybir.AluOpType.is_ge,
                )
                if nfull != ngrp:  # ragged tail group, column-by-column
                    for l in range(nfull * GROUP, ncols):
                        nc.vector.tensor_tensor(
                            out=cc[:, l // GROUP, 1, l % GROUP : l % GROUP + 1],
                            in0=tp_tile[:, a + l : a + l + 1],
                            in1=cc[:, l // GROUP, 0, l % GROUP : l % GROUP + 1],
                            op=mybir.AluOpType.is_ge,
                        )
                # cumulative >=-edge indicators
                for b in range(10):
                    nc.vector.tensor_scalar(
                        out=g[:, :ngrp, b, :],
                        in0=cc[:, :, 0, :],
                        scalar1=EDGES[b],
                        scalar2=None,
                        op0=mybir.AluOpType.is_ge,
                    )
                # one matmul per GROUP columns: psum[(a,i),(b,j)] accumulates
                # sum_p cc[p,a,gi*G+i] * g[p,b,gi*G+j]; host reads i==j blocks
                for gi in range(ngrp):
                    gg = group_base[c] + gi
                    nc.tensor.matmul(
                        psum[:],
                        lhsT=cc[:, gi, :, :],
                        rhs=g[:, gi, :, :],
                        start=(gg == 0),
                        stop=(gg == NGROUPS - 1),
                    )

            # supertile 1 first: its transfer overlaps the quarter DMAs
            st_tiles = {}

            def load_st(si):
                t = inbuf.tile([128, 16, C], f32, name="xt", tag="xt")
                eng = nc.sync if si % 2 == 0 else nc.scalar
                eng.dma_start(out=t[:], in_=xr[si])
                st_tiles[si] = t

            load_st(1)

            # supertile 0 split into quarter-DMAs so compute starts early
            for qi in range(4):
                q = inbuf.tile(
                    [128, 4, C], f32, name=f"q{qi}", tag=f"q{qi}", bufs=1
                )
                nc.sync.dma_start(out=q[:], in_=xr[0][:, 4 * qi : 4 * qi + 4, :])
                qs = mxbuf.tile(
                    [128, 4, H], f32, name=f"mq{qi}", tag=f"mq{qi}", bufs=1
                )
                emit_pmax(q[:], 4 * qi, 4, qs)

            nc.sync.dma_start(out=tp_tile[:], in_=tp[:])

            fired = [0]
            for si in (2, 3, 4):
                load_st(si)
            for s in range(1, NST):
                xt = st_tiles.pop(s)
                if s + 4 < NST:
                    load_st(s + 4)
                scratch = mxbuf.tile([128, 16, H], f32, name="mx", tag="mx")
                emit_pmax(xt[:], s * 16, 16, scratch)
                if s == 60:
                    # tail tile (rows 249856:249984) -> column 1952 and the
                    # 16-row partial -> column 1953; mid-stream, away from
                    # the busy startup and shutdown queues
                    xt2 = tailbuf.tile([128, 1, C], f32, name="xt2", tag="xt2")
                    nc.sync.dma_start(
                        out=xt2[:, 0, :],
                        in_=x_ap[NST * ST_ROWS : NST * ST_ROWS + 128, :],
                    )
                    ms2 = tailbuf.tile([128, 1, H], f32, name="ms2", tag="ms2")
                    emit_pmax(xt2[:], NT_MAIN, 1, ms2)
                    xt3 = tailbuf.tile([PARTIAL_ROWS, 1, C], f32, name="xt3", tag="xt3")
                    nc.sync.dma_start(
                        out=xt3[:, 0, :], in_=x_ap[NST * ST_ROWS + 128 :, :]
                    )
                    ms3 = tailbuf.tile(
                        [PARTIAL_ROWS, 1, H], f32, name="ms3", tag="ms3"
                    )
                    emit_pmax(xt3[:], NT_MAIN + 1, 1, ms3, nparts=PARTIAL_ROWS)
                done = (s + 1) * 16
                while (
                    fired[0] < NCHUNKS - 1
                    and CHUNK_STARTS[fired[0]] + CHUNK_SIZES[fired[0]] + 32 <= done
                ):
                    emit_chunk_epilogue(fired[0])
                    fired[0] += 1

            while fired[0] < NCHUNKS:
                emit_chunk_epilogue(fired[0])
                fired[0] += 1

            res = persist.tile([2 * GROUP, 10 * GROUP], f32)
            nc.vector.tensor_copy(out=res[:], in_=psum[:])
            nc.sync.dma_start(out=out[:], in_=res[:])

    nc.finalize()
    _NC_CACHE = nc
    return nc


def _prep_tprob(tpv: np.ndarray) -> np.ndarray:
    """[ROWS] f32 target-probs -> [128, NTG] f32, laid out per column.
    Unused slots get -1 so the on-chip correct flag [tprob >= conf]
    is 0 for phantom rows (their conf is memset to 0)."""
    tg = np.full((128, NTG), -1.0, dtype=np.float32)
    main = tpv[: NST * ST_ROWS].reshape(NST, 128, 16)
    tg[:, :NT_MAIN] = main.transpose(1, 0, 2).reshape(128, NT_MAIN)
    tg[:, NT_MAIN] = tpv[NST * ST_ROWS : NST * ST_ROWS + 128]
    tg[:PARTIAL_ROWS, NT_MAIN + 1] = tpv[NST * ST_ROWS + 128 :]
    return tg


def _run(inputs: np.ndarray, targets: np.ndarray, trace: bool = False):
    from concourse.bass_utils import run_bass_kernel_spmd

    nc = _build_bass()

    inputs = np.ascontiguousarray(inputs, dtype=np.float32)
    targets = np.asarray(targets)
    tprob = inputs[np.arange(inputs.shape[0]), targets.astype(np.int64)]

    in_maps = []
    for k in range(NCORES):
        lo = k * ROWS
        xs = inputs[lo : lo + ROWS]
        tpc = _prep_tprob(tprob[lo : lo + ROWS])
        in_maps.append({"x": xs, "tp": tpc})

    last_err = None
    for _attempt in range(3):
        try:
            r = run_bass_kernel_spmd(
                nc, in_maps, core_ids=list(range(NCORES)), trace=trace
            )
            break
        except Exception as e:  # transient NRT_EXEC_UNIT_UNRECOVERABLE on cold device
            last_err = e
    else:
        raise last_err
    return r


def _combine(results) -> np.ndarray:
    S = np.zeros((2, 10), dtype=np.float64)
    for r in results:
        o = r["out"].astype(np.float64).reshape(2, GROUP, 10, GROUP)
        S += np.einsum("aibi->ab", o)
    # S[a][b] = sum over rows with conf >= e_b; difference adjacent bins
    conf_sum = S[0] - np.append(S[0][1:], 0.0)
    corr_sum = S[1] - np.append(S[1][1:], 0.0)
    ece = np.abs(corr_sum - conf_sum).sum() / N
    return np.asarray(ece, dtype=np.float32)


def kernel(inputs: np.ndarray, targets: np.ndarray) -> np.ndarray:
    r = _run(inputs, targets, trace=False)
    return _combine(r.results)


# revision 28
# speedup vs baseline: 1.0270x; 1.0270x over previous
"""Expected Calibration Error (ECE) kernel for Trainium2, 8 NeuronCores.

Problem: inputs [2e6, 128] f32 row-probabilities, targets [2e6] int64.
  conf_i = max_c inputs[i, c];  pred_i = argmax_c inputs[i, c]
  bin_i  = bucketize(conf_i, linspace(0, 1, 11), right=True) - 1
  ECE    = sum_b |corr_sum[b] - conf_sum[b]| / N

Strategy (data-parallel over rows, 250k rows per core):
  The confidence per row is a pure paged max: one custom DVE op per
  2048-row supertile streams in0 = [128, 16, 128] (16 rows per
  partition) and computes an inclusive MAX-scan that RESETS at each
  row (page) boundary via a hand-crafted SUB_DIM_DONE step uop.  The
  page-end element of the scan is that row's conf; the (otherwise
  idle) Scalar engine extracts column 127 of each page into a packed
  conf tile.  This amortizes the ~115ns fixed DVE instruction cost
  over 2048 elements instead of 128, cutting Vector busy ~2x below
  the HBM roofline so the kernel is DMA-bound (the memory minimum).

  Correctness per row needs no argmax: the host precomputes
  tprob[i] = inputs[i, targets[i]] (gather) and ships it in the same
  [128, NTG] column layout as conf; on-chip correct = [tprob >= conf]
  (equality iff the target attains the row max; exact-f32-tie rows are
  ~1e-5 of data and ignorable).

  Binning: G_b = [conf >= e_b] for the 10 edges gives cumulative
  per-bin sums via tiny TensorE matmuls psum[2,10] += [conf,correct]^T
  x G per column; host differences adjacent bins and finishes
  sum |corr - conf| / N.

Sharding: rows split evenly, 250,000 per core = 122 supertiles x 16
tiles (p-major contiguous DMA) + 1 plain tile + 1 partial 16-row tile.
"""

import numpy as np

N = 2_000_000
C = 128
NCORES = 8
ROWS = N // NCORES            # 250_000
NST = 122                     # supertiles of 16 pages (2048 rows each)
ST_PAGES = 16
ST_ROWS = 128 * ST_PAGES      # 2048
NT_MAIN = NST * ST_PAGES      # 1952 conf columns via supertiles
# column 1952: full 128-row tile; column 1953: 16-row partial tile
NTG = NT_MAIN + 2             # 1954 conf columns
PARTIAL_ROWS = ROWS - NST * ST_ROWS - 128  # 16

# conf columns per epilogue/matmul chunk; chunk boundaries must be
# multiples of 16 (a supertile writes 16 columns in one op); smaller
# chunks at the end shrink the serial tail after the last supertile
CHUNK_SIZES = [256] * 7 + [64, 64, 34]
assert sum(CHUNK_SIZES) == NTG
CHUNK_STARTS = [sum(CHUNK_SIZES[:i]) for i in range(len(CHUNK_SIZES))]
NCHUNKS = len(CHUNK_SIZES)
for _s in CHUNK_STARTS:
    assert _s % 16 == 0

# columns per matmul: each matmul contracts GROUP columns at once into
# a [2*GROUP, 10*GROUP] PSUM accumulator (the host reads the diagonal
# sub-blocks); 16x fewer TensorE instructions than per-column matmuls,
# whose instruction-stream fetch DMAs congest queue 0
GROUP = 16
# last chunk padded to a GROUP multiple; pad cols are zeroed
CHUNK_PADS = [-(-sz // GROUP) * GROUP for sz in CHUNK_SIZES]
NGROUPS = sum(p // GROUP for p in CHUNK_PADS)

EDGES = [float(e) for e in np.linspace(0.0, 1.0, 11).astype(np.float32)[:10]]

_f32 = np.float32


def _paged_max_ref(in0, in1, c0, c1, c2):
    a = np.asarray(in0, np.float32)
    b = np.asarray(in1, np.float32).reshape(a.shape)
    m = np.maximum(a, b)
    if m.ndim == 2:
        m = m[:, None, :]
    # out gated to the last element of each page (write_subdim_last)
    return m.max(axis=-1)


def _register_op():
    import copy

    from concourse.dve_ops import (
        DveOp,
        OPS,
        CUSTOM_DVE_SPECS,
        _SUB_OPCODE_FOR_NAME,
        _CUSTOM_DVE_ROW_BASE,
        _COMPILE_CACHE,
    )
    from concourse.dve_spec import Spec, Src0, Src1, MaxNeg, maxx, scan, lower, AluOp
    from concourse.dve_uop import DveOpSpec, Trigger, AluInp, ENABLE

    name = "ECE_PAGED_MAX_ANT"
    if name in _SUB_OPCODE_FOR_NAME:
        return next(op for op in OPS if op.name == name)

    # two streams (half a row each) so the DVE eats 2 classes/cycle
    spec = Spec(
        body=scan(AluOp.MAX, maxx(Src0, Src1), init=MaxNeg),
        reference=_paged_max_ref,
    )
    row = _CUSTOM_DVE_ROW_BASE + len(OPS)
    assert row < 0x20
    _SUB_OPCODE_FOR_NAME[name] = row

    shas = {}
    for ver in ("v3", "v4"):
        try:
            seed, steady = lower(spec, ver=ver)
        except Exception:
            continue
        # locate the scan stage (MAX with same-stage CURR feedback)
        ss = next(
            i
            for i, b in enumerate(steady.datapath_config)
            if b.op == AluOp.MAX and b.alu_src0 == AluInp.CURR_ALU_OUT
        )
        src0_inp = steady.datapath_config[ss].alu_src1  # the Src0 lane
        # steady: on each page boundary jump to the step uop (index 2)
        steady.trigger = (
            Trigger.SRC_TENSOR_DONE,
            Trigger.SUB_DIM_DONE,
            Trigger.NONE,
        )
        steady.next_uop = (0, 2, 0)
        # write the scan value only at the last element of each page --
        # one conf per row, straight into the packed conf tile
        steady.out_last_subdim_enable = ENABLE
        # step: first element of a new page resets the running max to
        # that element (BYPASS of Src0 instead of MAX with the carry)
        step = copy.deepcopy(steady)
        dp = step.datapath_config[ss]
        dp.op = AluOp.BYPASS
        dp.alu_src0 = src0_inp
        dp.alu_src1 = src0_inp
        step.trigger = (
            Trigger.SRC_TENSOR_DONE,
            Trigger.SUB_DIM_DONE,
            Trigger.COUNT,
        )
        step.next_uop = (0, 2, 1)
        step.repeat_count = 1
        uops = [seed, steady, step]
        dspec = DveOpSpec(name=name, opcode=row, uops=uops, rd1_en=True)
        dspec.validate(ver)
        _COMPILE_CACHE[(name, ver)] = dspec
        shas[ver] = dspec.sha(ver)
    op = DveOp(name, spec, subdim=True, uops_sha=shas)
    OPS.append(op)
    CUSTOM_DVE_SPECS[name] = spec
    return op


_NC_CACHE = None


def _build_bass():
    global _NC_CACHE
    if _NC_CACHE is not None:
        return _NC_CACHE

    import concourse.bacc as bacc
    import concourse.tile as tile
    from concourse import mybir

    pmax_op = _register_op()

    nc = bacc.Bacc()
    f32 = mybir.dt.float32
    x = nc.dram_tensor("x", [ROWS, C], f32, kind="ExternalInput")
    tp = nc.dram_tensor("tp", [128, NTG], f32, kind="ExternalInput")
    out = nc.dram_tensor("out", [2 * GROUP, 10 * GROUP], f32, kind="ExternalOutput")

    with tile.TileContext(nc) as tc:
        with (
            tc.tile_pool(name="persist", bufs=1) as persist,
            tc.tile_pool(name="inbuf", bufs=6) as inbuf,
            tc.tile_pool(name="tailbuf", bufs=1) as tailbuf,
            tc.tile_pool(name="decbuf", bufs=3) as decbuf,
            tc.tile_pool(name="psum", bufs=1, space="PSUM") as psumpool,
        ):
            # [128, NTG] target-prob tile, loaded as one 1MB transfer so
            # the DMA splits evenly across all 16 queues (chunked 128KB
            # loads all land on one queue and make it a straggler)
            tp_tile = persist.tile([128, NTG], f32, name="tpt", tag="tpt")

            # per-chunk [conf, correct] tiles, group-major so each matmul
            # reads a contiguous [128, 2*GROUP] block: (p, group, a, j);
            # conf (a=0) is written one group at a time by the extracts
            cc_tiles = [
                persist.tile(
                    [128, CHUNK_PADS[c] // GROUP, 2, GROUP],
                    f32,
                    name=f"cc{c}",
                    tag=f"cc{c}",
                )
                for c in range(NCHUNKS)
            ]
            # partial-tile column: partitions 16.. of the last column are
            # never written, and pad columns are never written; conf=0
            # there (with tprob=-1) zeroes their contribution
            nc.vector.memset(cc_tiles[-1][:], 0.0)

            psum = psumpool.tile([2 * GROUP, 10 * GROUP], f32)

            x_ap = x[:]
            xr = x_ap[: NST * ST_ROWS, :].rearrange(
                "(s p k) c -> s p k c", s=NST, p=128, k=16
            )

            import bisect

            def conf_dst(j, npages, nparts=128):
                """conf destination AP for columns [j, j+npages)."""
                c = bisect.bisect_right(CHUNK_STARTS, j) - 1
                l = j - CHUNK_STARTS[c]
                gi, jo = l // GROUP, l % GROUP
                assert jo + npages <= GROUP
                return cc_tiles[c][:nparts, gi, 0, jo : jo + npages]

            H = C // 2

            def emit_pmax(xt_ap, j, npages, nparts=128):
                """paged-max over xt [nparts, npages, C]; conf -> cols j.."""
                nc.vector._custom_dve(
                    pmax_op,
                    out=conf_dst(j, npages, nparts),
                    in0=xt_ap[:, :, :H],
                    in1=xt_ap[:, :, H:],
                )

            group_base = [sum(p // GROUP for p in CHUNK_PADS[:c]) for c in range(NCHUNKS)]

            def emit_chunk_epilogue(c):
                ncols = CHUNK_SIZES[c]
                npad = CHUNK_PADS[c]
                ngrp = npad // GROUP
                nfull = ncols // GROUP  # groups fully covered by real cols
                a = CHUNK_STARTS[c]
                cc = cc_tiles[c]
                if npad != ncols:
                    g = decbuf.tile(
                        [128, ngrp, 10, GROUP], f32, name=f"g2_{c}", tag=f"g2_{c}", bufs=1
                    )
                    nc.vector.memset(g[:], 0.0)
                else:
                    g = decbuf.tile([128, 16, 10, GROUP], f32, name="g", tag="g")
                tpr = tp_tile[:, a : a + nfull * GROUP].rearrange(
                    "p (g j) -> p g j", g=nfull
                )
                # correct flag: [tprob >= conf]
                nc.vector.tensor_tensor(
                    out=cc[:, :nfull, 1, :],
                    in0=tpr,
                    in1=cc[:, :nfull, 0, :],
                    op=mybir.AluOpType.is_ge,
                )
                if nfull != ngrp:  # ragged tail group, column-by-column
                    for l in range(nfull * GROUP, ncols):
                        nc.vector.tensor_tensor(
                            out=cc[:, l // GROUP, 1, l % GROUP : l % GROUP + 1],
                            in0=tp_tile[:, a + l : a + l + 1],
                            in1=cc[:, l // GROUP, 0, l % GROUP : l % GROUP + 1],
                            op=mybir.AluOpType.is_ge,
                        )
                # cumulative >=-edge indicators
                for b in range(10):
                    nc.vector.tensor_scalar(
                        out=g[:, :ngrp, b, :],
                        in0=cc[:, :, 0, :],
                        scalar1=EDGES[b],
                        scalar2=None,
                        op0=mybir.AluOpType.is_ge,
                    )
                # one matmul per GROUP columns: psum[(a,i),(b,j)] accumulates
                # sum_p cc[p,a,gi*G+i] * g[p,b,gi*G+j]; host reads i==j blocks
                for gi in range(ngrp):
                    gg = group_base[c] + gi
                    nc.tensor.matmul(
                        psum[:],
                        lhsT=cc[:, gi, :, :],
                        rhs=g[:, gi, :, :],
                        start=(gg == 0),
                        stop=(gg == NGROUPS - 1),
                    )

            # supertile 1 first: its transfer overlaps the quarter DMAs
            st_tiles = {}

            def load_st(si):
                t = inbuf.tile([128, 16, C], f32, name="xt", tag="xt")
                eng = nc.sync if si % 2 == 0 else nc.scalar
                eng.dma_start(out=t[:], in_=xr[si])
                st_tiles[si] = t

            load_st(1)

            # supertile 0 split into quarter-DMAs so compute starts early
            for qi in range(4):
                q = inbuf.tile(
                    [128, 4, C], f32, name=f"q{qi}", tag=f"q{qi}", bufs=1
                )
                nc.sync.dma_start(out=q[:], in_=xr[0][:, 4 * qi : 4 * qi + 4, :])
                emit_pmax(q[:], 4 * qi, 4)

            nc.sync.dma_start(out=tp_tile[:], in_=tp[:])

            fired = [0]
            for si in (2, 3, 4):
                load_st(si)
            for s in range(1, NST):
                xt = st_tiles.pop(s)
                if s + 4 < NST:
                    load_st(s + 4)
                emit_pmax(xt[:], s * 16, 16)
                if s == 60:
                    # tail tile (rows 249856:249984) -> column 1952 and the
                    # 16-row partial -> column 1953; mid-stream, away from
                    # the busy startup and shutdown queues
                    xt2 = tailbuf.tile([128, 1, C], f32, name="xt2", tag="xt2")
                    nc.sync.dma_start(
                        out=xt2[:, 0, :],
                        in_=x_ap[NST * ST_ROWS : NST * ST_ROWS + 128, :],
                    )
                    emit_pmax(xt2[:], NT_MAIN, 1)
                    xt3 = tailbuf.tile([PARTIAL_ROWS, 1, C], f32, name="xt3", tag="xt3")
                    nc.sync.dma_start(
                        out=xt3[:, 0, :], in_=x_ap[NST * ST_ROWS + 128 :, :]
                    )
                    emit_pmax(xt3[:], NT_MAIN + 1, 1, nparts=PARTIAL_ROWS)
                done = (s + 1) * 16
                while (
                    fired[0] < NCHUNKS - 1
                    and CHUNK_STARTS[fired[0]] + CHUNK_SIZES[fired[0]] + 32 <= done
                ):
                    emit_chunk_epilogue(fired[0])
                    fired[0] += 1

            while fired[0] < NCHUNKS:
                emit_chunk_epilogue(fired[0])
                fired[0] += 1

            res = persist.tile([2 * GROUP, 10 * GROUP], f32)
            nc.vector.tensor_copy(out=res[:], in_=psum[:])
            nc.sync.dma_start(out=out[:], in_=res[:])

    nc.finalize()
    _NC_CACHE = nc
    return nc


def _prep_tprob(tpv: np.ndarray) -> np.ndarray:
    """[ROWS] f32 target-probs -> [128, NTG] f32, laid out per column.
    Unused slots get -1 so the on-chip correct flag [tprob >= conf]
    is 0 for phantom rows (their conf is memset to 0)."""
    tg = np.full((128, NTG), -1.0, dtype=np.float32)
    main = tpv[: NST * ST_ROWS].reshape(NST, 128, 16)
    tg[:, :NT_MAIN] = main.transpose(1, 0, 2).reshape(128, NT_MAIN)
    tg[:, NT_MAIN] = tpv[NST * ST_ROWS : NST * ST_ROWS + 128]
    tg[:PARTIAL_ROWS, NT_MAIN + 1] = tpv[NST * ST_ROWS + 128 :]
    return tg


def _run(inputs: np.ndarray, targets: np.ndarray, trace: bool = False):
    from concourse.bass_utils import run_bass_kernel_spmd

    nc = _build_bass()

    inputs = np.ascontiguousarray(inputs, dtype=np.float32)
    targets = np.asarray(targets)
    tprob = inputs[np.arange(inputs.shape[0]), targets.astype(np.int64)]

    in_maps = []
    for k in range(NCORES):
        lo = k * ROWS
        xs = inputs[lo : lo + ROWS]
        tpc = _prep_tprob(tprob[lo : lo + ROWS])
        in_maps.append({"x": xs, "tp": tpc})

    last_err = None
    for _attempt in range(3):
        try:
            r = run_bass_kernel_spmd(
                nc, in_maps, core_ids=list(range(NCORES)), trace=trace
            )
            break
        except Exception as e:  # transient NRT_EXEC_UNIT_UNRECOVERABLE on cold device
            last_err = e
    else:
        raise last_err
    return r


def _combine(results) -> np.ndarray:
    S = np.zeros((2, 10), dtype=np.float64)
    for r in results:
        o = r["out"].astype(np.float64).reshape(2, GROUP, 10, GROUP)
        S += np.einsum("aibi->ab", o)
    # S[a][b] = sum over rows with conf >= e_b; difference adjacent bins
    conf_sum = S[0] - np.append(S[0][1:], 0.0)
    corr_sum = S[1] - np.append(S[1][1:], 0.0)
    ece = np.abs(corr_sum - conf_sum).sum() / N
    return np.asarray(ece, dtype=np.float32)


def kernel(inputs: np.ndarray, targets: np.ndarray) -> np.ndarray:
    r = _run(inputs, targets, trace=False)
    return _combine(r.results)


# revision 29
# speedup vs baseline: 1.2286x; 1.1964x over previous
"""Expected Calibration Error (ECE) kernel for Trainium2, 8 NeuronCores.

Problem: inputs [2e6, 128] f32 row-probabilities, targets [2e6] int64.
  conf_i = max_c inputs[i, c];  pred_i = argmax_c inputs[i, c]
  bin_i  = bucketize(conf_i, linspace(0, 1, 11), right=True) - 1
  ECE    = sum_b |corr_sum[b] - conf_sum[b]| / N

Strategy (data-parallel over rows, 250k rows per core):
  The confidence per row is a pure paged max: one custom DVE op per
  2048-row supertile streams in0 = [128, 16, 128] (16 rows per
  partition) and computes an inclusive MAX-scan that RESETS at each
  row (page) boundary via a hand-crafted SUB_DIM_DONE step uop.  The
  page-end element of the scan is that row's conf; the (otherwise
  idle) Scalar engine extracts column 127 of each page into a packed
  conf tile.  This amortizes the ~115ns fixed DVE instruction cost
  over 2048 elements instead of 128, cutting Vector busy ~2x below
  the HBM roofline so the kernel is DMA-bound (the memory minimum).

  Correctness per row needs no argmax: the host precomputes
  tprob[i] = inputs[i, targets[i]] (gather) and ships it in the same
  [128, NTG] column layout as conf; on-chip correct = [tprob >= conf]
  (equality iff the target attains the row max; exact-f32-tie rows are
  ~1e-5 of data and ignorable).

  Binning: G_b = [conf >= e_b] for the 10 edges gives cumulative
  per-bin sums via tiny TensorE matmuls psum[2,10] += [conf,correct]^T
  x G per column; host differences adjacent bins and finishes
  sum |corr - conf| / N.

Sharding: rows split evenly, 250,000 per core = 122 supertiles x 16
tiles (p-major contiguous DMA) + 1 plain tile + 1 partial 16-row tile.
"""

import numpy as np

N = 2_000_000
C = 128
NCORES = 8
ROWS = N // NCORES            # 250_000
NST = 122                     # supertiles of 16 pages (2048 rows each)
ST_PAGES = 16
ST_ROWS = 128 * ST_PAGES      # 2048
NT_MAIN = NST * ST_PAGES      # 1952 conf columns via supertiles
# column 1952: full 128-row tile; column 1953: 16-row partial tile
NTG = NT_MAIN + 2             # 1954 conf columns
PARTIAL_ROWS = ROWS - NST * ST_ROWS - 128  # 16

# conf columns per epilogue/matmul chunk; chunk boundaries must be
# multiples of 16 (a supertile writes 16 columns in one op); smaller
# chunks at the end shrink the serial tail after the last supertile
CHUNK_SIZES = [256] * 7 + [64, 64, 34]
assert sum(CHUNK_SIZES) == NTG
CHUNK_STARTS = [sum(CHUNK_SIZES[:i]) for i in range(len(CHUNK_SIZES))]
NCHUNKS = len(CHUNK_SIZES)
for _s in CHUNK_STARTS:
    assert _s % 16 == 0

# columns per matmul: each matmul contracts GROUP columns at once into
# a [2*GROUP, 10*GROUP] PSUM accumulator (the host reads the diagonal
# sub-blocks); 16x fewer TensorE instructions than per-column matmuls,
# whose instruction-stream fetch DMAs congest queue 0
GROUP = 16
# last chunk padded to a GROUP multiple; pad cols are zeroed
CHUNK_PADS = [-(-sz // GROUP) * GROUP for sz in CHUNK_SIZES]
NGROUPS = sum(p // GROUP for p in CHUNK_PADS)

EDGES = [float(e) for e in np.linspace(0.0, 1.0, 11).astype(np.float32)[:10]]

_f32 = np.float32


def _paged_max_ref(in0, in1, c0, c1, c2):
    m = np.asarray(in0, np.float32)
    if m.ndim == 2:
        m = m[:, None, :]
    # out gated to the last element of each page (write_subdim_last)
    return m.max(axis=-1)


def _register_op():
    import copy

    from concourse.dve_ops import (
        DveOp,
        OPS,
        CUSTOM_DVE_SPECS,
        _SUB_OPCODE_FOR_NAME,
        _CUSTOM_DVE_ROW_BASE,
        _COMPILE_CACHE,
    )
    from concourse.dve_spec import Spec, Src0, Src1, MaxNeg, maxx, scan, lower, AluOp
    from concourse.dve_uop import DveOpSpec, Trigger, AluInp, ENABLE

    name = "ECE_PAGED_MAX_ANT"
    if name in _SUB_OPCODE_FOR_NAME:
        return next(op for op in OPS if op.name == name)

    spec = Spec(
        body=scan(AluOp.MAX, Src0, init=MaxNeg),
        reference=_paged_max_ref,
    )
    row = _CUSTOM_DVE_ROW_BASE + len(OPS)
    assert row < 0x20
    _SUB_OPCODE_FOR_NAME[name] = row

    shas = {}
    for ver in ("v3", "v4"):
        try:
            seed, steady = lower(spec, ver=ver)
        except Exception:
            continue
        # locate the scan stage (MAX with same-stage CURR feedback)
        ss = next(
            i
            for i, b in enumerate(steady.datapath_config)
            if b.op == AluOp.MAX and b.alu_src0 == AluInp.CURR_ALU_OUT
        )
        src0_inp = steady.datapath_config[ss].alu_src1  # the Src0 lane
        # steady: on each page boundary jump to the step uop (index 2)
        steady.trigger = (
            Trigger.SRC_TENSOR_DONE,
            Trigger.SUB_DIM_DONE,
            Trigger.NONE,
        )
        steady.next_uop = (0, 2, 0)
        # write the scan value only at the last element of each page --
        # one conf per row, straight into the packed conf tile
        steady.out_last_subdim_enable = ENABLE
        # step: first element of a new page resets the running max to
        # that element (BYPASS of Src0 instead of MAX with the carry)
        step = copy.deepcopy(steady)
        dp = step.datapath_config[ss]
        dp.op = AluOp.BYPASS
        dp.alu_src0 = src0_inp
        dp.alu_src1 = src0_inp
        step.trigger = (
            Trigger.SRC_TENSOR_DONE,
            Trigger.SUB_DIM_DONE,
            Trigger.COUNT,
        )
        step.next_uop = (0, 2, 1)
        step.repeat_count = 1
        uops = [seed, steady, step]
        dspec = DveOpSpec(name=name, opcode=row, uops=uops, rd1_en=False)
        dspec.validate(ver)
        _COMPILE_CACHE[(name, ver)] = dspec
        shas[ver] = dspec.sha(ver)
    op = DveOp(name, spec, subdim=True, uops_sha=shas)
    OPS.append(op)
    CUSTOM_DVE_SPECS[name] = spec
    return op


_NC_CACHE = None


def _build_bass():
    global _NC_CACHE
    if _NC_CACHE is not None:
        return _NC_CACHE

    import concourse.bacc as bacc
    import concourse.tile as tile
    from concourse import mybir

    pmax_op = _register_op()

    nc = bacc.Bacc()
    f32 = mybir.dt.float32
    x = nc.dram_tensor("x", [ROWS, C], f32, kind="ExternalInput")
    tp = nc.dram_tensor("tp", [128, NTG], f32, kind="ExternalInput")
    out = nc.dram_tensor("out", [2 * GROUP, 10 * GROUP], f32, kind="ExternalOutput")

    with tile.TileContext(nc) as tc:
        with (
            tc.tile_pool(name="persist", bufs=1) as persist,
            tc.tile_pool(name="inbuf", bufs=6) as inbuf,
            tc.tile_pool(name="tailbuf", bufs=1) as tailbuf,
            tc.tile_pool(name="decbuf", bufs=3) as decbuf,
            tc.tile_pool(name="psum", bufs=1, space="PSUM") as psumpool,
        ):
            # [128, NTG] target-prob tile, loaded as one 1MB transfer so
            # the DMA splits evenly across all 16 queues (chunked 128KB
            # loads all land on one queue and make it a straggler)
            tp_tile = persist.tile([128, NTG], f32, name="tpt", tag="tpt")

            # per-chunk [conf, correct] tiles, group-major so each matmul
            # reads a contiguous [128, 2*GROUP] block: (p, group, a, j);
            # conf (a=0) is written one group at a time by the extracts
            cc_tiles = [
                persist.tile(
                    [128, CHUNK_PADS[c] // GROUP, 2, GROUP],
                    f32,
                    name=f"cc{c}",
                    tag=f"cc{c}",
                )
                for c in range(NCHUNKS)
            ]
            # partial-tile column: partitions 16.. of the last column are
            # never written, and pad columns are never written; conf=0
            # there (with tprob=-1) zeroes their contribution
            nc.vector.memset(cc_tiles[-1][:], 0.0)

            psum = psumpool.tile([2 * GROUP, 10 * GROUP], f32)

            x_ap = x[:]
            xr = x_ap[: NST * ST_ROWS, :].rearrange(
                "(s p k) c -> s p k c", s=NST, p=128, k=16
            )

            import bisect

            def conf_dst(j, npages, nparts=128):
                """conf destination AP for columns [j, j+npages)."""
                c = bisect.bisect_right(CHUNK_STARTS, j) - 1
                l = j - CHUNK_STARTS[c]
                gi, jo = l // GROUP, l % GROUP
                assert jo + npages <= GROUP
                return cc_tiles[c][:nparts, gi, 0, jo : jo + npages]

            H = C // 2

            def emit_pmax(xt_ap, j, npages, nparts=128):
                """paged-max over xt [nparts, npages, C]; conf -> cols j.."""
                nc.vector._custom_dve(
                    pmax_op,
                    out=conf_dst(j, npages, nparts),
                    in0=xt_ap,
                )

            group_base = [sum(p // GROUP for p in CHUNK_PADS[:c]) for c in range(NCHUNKS)]

            def emit_chunk_epilogue(c):
                ncols = CHUNK_SIZES[c]
                npad = CHUNK_PADS[c]
                ngrp = npad // GROUP
                nfull = ncols // GROUP  # groups fully covered by real cols
                a = CHUNK_STARTS[c]
                cc = cc_tiles[c]
                if npad != ncols:
                    g = decbuf.tile(
                        [128, ngrp, 10, GROUP], f32, name=f"g2_{c}", tag=f"g2_{c}", bufs=1
                    )
                    nc.vector.memset(g[:], 0.0)
                else:
                    g = decbuf.tile([128, 16, 10, GROUP], f32, name="g", tag="g")
                tpr = tp_tile[:, a : a + nfull * GROUP].rearrange(
                    "p (g j) -> p g j", g=nfull
                )
                # correct flag: [tprob >= conf]
                nc.vector.tensor_tensor(
                    out=cc[:, :nfull, 1, :],
                    in0=tpr,
                    in1=cc[:, :nfull, 0, :],
                    op=mybir.AluOpType.is_ge,
                )
                if nfull != ngrp:  # ragged tail group, column-by-column
                    for l in range(nfull * GROUP, ncols):
                        nc.vector.tensor_tensor(
                            out=cc[:, l // GROUP, 1, l % GROUP : l % GROUP + 1],
                            in0=tp_tile[:, a + l : a + l + 1],
                            in1=cc[:, l // GROUP, 0, l % GROUP : l % GROUP + 1],
                            op=mybir.AluOpType.is_ge,
                        )
                # cumulative >=-edge indicators
                for b in range(10):
                    nc.vector.tensor_scalar(
                        out=g[:, :ngrp, b, :],
                        in0=cc[:, :, 0, :],
                        scalar1=EDGES[b],
                        scalar2=None,
                        op0=mybir.AluOpType.is_ge,
                    )
                # one matmul per GROUP columns: psum[(a,i),(b,j)] accumulates
                # sum_p cc[p,a,gi*G+i] * g[p,b,gi*G+j]; host reads i==j blocks
                for gi in range(ngrp):
                    gg = group_base[c] + gi
                    nc.tensor.matmul(
                        psum[:],
                        lhsT=cc[:, gi, :, :],
                        rhs=g[:, gi, :, :],
                        start=(gg == 0),
                        stop=(gg == NGROUPS - 1),
                    )

            # supertile 1 first: its transfer overlaps the quarter DMAs
            st_tiles = {}

            def load_st(si):
                t = inbuf.tile([128, 16, C], f32, name="xt", tag="xt")
                eng = nc.sync if si % 2 == 0 else nc.scalar
                eng.dma_start(out=t[:], in_=xr[si])
                st_tiles[si] = t

            load_st(1)

            # supertile 0 split into quarter-DMAs so compute starts early
            for qi in range(4):
                q = inbuf.tile(
                    [128, 4, C], f32, name=f"q{qi}", tag=f"q{qi}", bufs=1
                )
                nc.sync.dma_start(out=q[:], in_=xr[0][:, 4 * qi : 4 * qi + 4, :])
                emit_pmax(q[:], 4 * qi, 4)

            nc.sync.dma_start(out=tp_tile[:], in_=tp[:])

            fired = [0]
            for si in (2, 3, 4):
                load_st(si)
            for s in range(1, NST):
                xt = st_tiles.pop(s)
                if s + 4 < NST:
                    load_st(s + 4)
                emit_pmax(xt[:], s * 16, 16)
                if s == 60:
                    # tail tile (rows 249856:249984) -> column 1952 and the
                    # 16-row partial -> column 1953; mid-stream, away from
                    # the busy startup and shutdown queues
                    xt2 = tailbuf.tile([128, 1, C], f32, name="xt2", tag="xt2")
                    nc.sync.dma_start(
                        out=xt2[:, 0, :],
                        in_=x_ap[NST * ST_ROWS : NST * ST_ROWS + 128, :],
                    )
                    emit_pmax(xt2[:], NT_MAIN, 1)
                    xt3 = tailbuf.tile([PARTIAL_ROWS, 1, C], f32, name="xt3", tag="xt3")
                    nc.sync.dma_start(
                        out=xt3[:, 0, :], in_=x_ap[NST * ST_ROWS + 128 :, :]
                    )
                    emit_pmax(xt3[:], NT_MAIN + 1, 1, nparts=PARTIAL_ROWS)
                done = (s + 1) * 16
                while (
                    fired[0] < NCHUNKS - 1
                    and CHUNK_STARTS[fired[0]] + CHUNK_SIZES[fired[0]] + 32 <= done
                ):
                    emit_chunk_epilogue(fired[0])
                    fired[0] += 1

            while fired[0] < NCHUNKS:
                emit_chunk_epilogue(fired[0])
                fired[0] += 1

            res = persist.tile([2 * GROUP, 10 * GROUP], f32)
            nc.vector.tensor_copy(out=res[:], in_=psum[:])
            nc.sync.dma_start(out=out[:], in_=res[:])

    nc.finalize()
    _NC_CACHE = nc
    return nc


def _prep_tprob(tpv: np.ndarray) -> np.ndarray:
    """[ROWS] f32 target-probs -> [128, NTG] f32, laid out per column.
    Unused slots get -1 so the on-chip correct flag [tprob >= conf]
    is 0 for phantom rows (their conf is memset to 0)."""
    tg = np.full((128, NTG), -1.0, dtype=np.float32)
    main = tpv[: NST * ST_ROWS].reshape(NST, 128, 16)
    tg[:, :NT_MAIN] = main.transpose(1, 0, 2).reshape(128, NT_MAIN)
    tg[:, NT_MAIN] = tpv[NST * ST_ROWS : NST * ST_ROWS + 128]
    tg[:PARTIAL_ROWS, NT_MAIN + 1] = tpv[NST * ST_ROWS + 128 :]
    return tg


def _run(inputs: np.ndarray, targets: np.ndarray, trace: bool = False):
    from concourse.bass_utils import run_bass_kernel_spmd

    nc = _build_bass()

    inputs = np.ascontiguousarray(inputs, dtype=np.float32)
    targets = np.asarray(targets)
    tprob = inputs[np.arange(inputs.shape[0]), targets.astype(np.int64)]

    in_maps = []
    for k in range(NCORES):
        lo = k * ROWS
        xs = inputs[lo : lo + ROWS]
        tpc = _prep_tprob(tprob[lo : lo + ROWS])
        in_maps.append({"x": xs, "tp": tpc})

    last_err = None
    for _attempt in range(3):
        try:
            r = run_bass_kernel_spmd(
                nc, in_maps, core_ids=list(range(NCORES)), trace=trace
            )
            break
        except Exception as e:  # transient NRT_EXEC_UNIT_UNRECOVERABLE on cold device
            last_err = e
    else:
        raise last_err
    return r


def _combine(results) -> np.ndarray:
    S = np.zeros((2, 10), dtype=np.float64)
    for r in results:
        o = r["out"].astype(np.float64).reshape(2, GROUP, 10, GROUP)
        S += np.einsum("aibi->ab", o)
    # S[a][b] = sum over rows with conf >= e_b; difference adjacent bins
    conf_sum = S[0] - np.append(S[0][1:], 0.0)
    corr_sum = S[1] - np.append(S[1][1:], 0.0)
    ece = np.abs(corr_sum - conf_sum).sum() / N
    return np.asarray(ece, dtype=np.float32)


def kernel(inputs: np.ndarray, targets: np.ndarray) -> np.ndarray:
    r = _run(inputs, targets, trace=False)
    return _combine(r.results)
